# revision 6
# baseline (speedup 1.0000x reference)
"""MoE layer (T=16384, H=1024, F=4096, E=8, top-2) on 8 Trainium2 cores.

Strategy (FFN-dim sharding — perfectly balanced):
  - Router (x @ Wg, softmax, top-2, renormalize) runs on host with jax-on-CPU
    so expert selection matches the reference bit-for-bit.
  - Tokens are gathered per expert into one concatenated sequence of
    column chunks (<=512 tokens, each chunk belonging to one expert, counts
    padded to a multiple of 4). The SAME token data ships to all 8 cores in
    a chunk-major layout xc[j] = [P, KT, n_j] so each chunk is ONE
    contiguous 1MB DMA (8KB per partition line).
  - Core c owns the F-slice [c*512, (c+1)*512) of ALL experts' w1/w2
    (16.8 MB bf16, SBUF-resident; preloaded on the scalar HWDGE ring so the
    token stream on the sync ring is not blocked behind it). It computes
        y_partial = silu(xe @ w1[e][:, sl]) @ w2[e][sl, :]
    for every chunk. Every core processes every token-expert pair ->
    identical work on all cores (zero load imbalance).
  - Host combine: y = sum over cores of partials (fp32), then scatter-add
    with the top-2 gate weights.
"""

import numpy as np
import ml_dtypes

T, H, F, E, TOPK = 16384, 1024, 4096, 8, 2
P = 128
NCORE = 8
FS = F // NCORE       # 512: per-core F-slice width
KT = H // P           # 8  k-tiles over H (GEMM1 contraction)
FT = FS // P          # 4  f-tiles per core slice
HT = H // P           # 8  output tiles over H
CHUNK = 512           # max matmul moving free dim (tokens per chunk)
PAD = 4               # per-expert token count padded to a multiple of this

BF16 = ml_dtypes.bfloat16

_module_cache: dict = {}


def _routing(x: np.ndarray, Wg: np.ndarray):
    """Top-2 expert ids and renormalized gates, matching the jax reference.

    The reference receives numpy arrays, so its `x @ Wg` runs through numpy
    BLAS — replicate that exactly (the expert ranking has 1-ulp knife-edge
    ties that flip between BLAS and XLA matmul). softmax/top_k then follow
    the reference's jax ops on CPU.
    """
    logits = x @ Wg  # numpy BLAS fp32, same as reference(**np_inputs)
    try:
        import jax
        import jax.numpy as jnp

        cpu = jax.devices("cpu")[0]
        with jax.default_device(cpu):
            lj = jax.device_put(jnp.asarray(logits), cpu)
            probs = jax.nn.softmax(lj, axis=-1)
            tv, ti = jax.lax.top_k(probs, TOPK)
            rw = tv / jnp.sum(tv, axis=-1, keepdims=True)
        return np.asarray(ti), np.asarray(rw, np.float32)
    except Exception:
        m = logits.max(axis=1, keepdims=True)
        p = np.exp(logits - m)
        p /= p.sum(axis=1, keepdims=True)
        order = np.argsort(-p, axis=1, kind="stable")
        ti = order[:, :TOPK]
        tv = np.take_along_axis(p, ti, axis=1)
        rw = (tv / tv.sum(axis=1, keepdims=True)).astype(np.float32)
        return ti, rw


def _chunk_plan(cnts_padded):
    """[(expert, col offset, n tokens)] covering each expert's padded range."""
    plan = []
    off = 0
    for e, c in enumerate(cnts_padded):
        rem = c
        while rem > 0:
            n = min(CHUNK, rem)
            plan.append((e, off, n))
            off += n
            rem -= n
    return tuple(plan)


def _build_module(plan, repeat: int = 1):
    """Bass/Tile module for one F-slice core.

    All weights (every expert's slice) are SBUF-resident; the token chunk
    loop runs GEMM1 (4 f-tiles x 8 k) -> silu -> GEMM2 (8 h-tiles x 4 f)
    with the chunk's expert baked in at compile time.
    """
    import concourse.bass as bass
    import concourse.mybir as mybir
    import concourse.tile as tile
    from concourse import bacc
    from concourse.bass import ts

    dt = mybir.dt
    nch = len(plan)
    W1SZ = KT * FS                  # 4096 columns of w1 slice per expert
    WSZ = W1SZ + FT * H             # + 4096 columns of w2 slice

    nc = bacc.Bacc("TRN2", target_bir_lowering=False, debug=False)

    xc = nc.dram_tensor("xc", (nch, P, KT, CHUNK), dt.bfloat16,
                        kind="ExternalInput").ap()
    wz = nc.dram_tensor("wz", (E, P, WSZ), dt.bfloat16,
                        kind="ExternalInput").ap()
    yc = nc.dram_tensor("yc", (nch, P, HT, CHUNK), dt.bfloat16,
                        kind="ExternalOutput").ap()

    with tile.TileContext(nc) as tc:
        with (
            tc.tile_pool(name="wpool", bufs=1) as wpool,
            tc.tile_pool(name="xpool", bufs=3) as xpool,
            tc.tile_pool(name="hpool", bufs=2) as hpool,
            tc.tile_pool(name="opool", bufs=2) as opool,
            tc.tile_pool(name="spool", bufs=2) as spool,
            tc.tile_pool(name="ps1", bufs=4, space="PSUM") as ps1,
            tc.tile_pool(name="ps2", bufs=4, space="PSUM") as ps2,
        ):
            # Resident weights: 128KB per partition, loaded via the GPSIMD
            # SWDGE queue so neither the sync ring (token chunks) nor the
            # scalar engine (sigmoids; its queue must not sit behind DMA
            # trigger instructions) is blocked. Expert 0 is split into
            # per-k-tile pieces so the first chunk's matmuls can start as
            # soon as the first 128KB lands.
            wzs = wpool.tile([P, E, WSZ], dt.bfloat16)
            for k in range(KT):
                nc.gpsimd.dma_start(
                    out=wzs[:, 0, k * FS : (k + 1) * FS],
                    in_=wz[0, :, k * FS : (k + 1) * FS],
                )
            for f in range(FT):
                a = W1SZ + f * H
                nc.gpsimd.dma_start(
                    out=wzs[:, 0, a : a + H], in_=wz[0, :, a : a + H]
                )
            nc.gpsimd.dma_start(out=wzs[:, 1, :W1SZ], in_=wz[1, :, :W1SZ])
            nc.gpsimd.dma_start(out=wzs[:, 1, W1SZ:], in_=wz[1, :, W1SZ:])
            for e in range(2, E):
                nc.gpsimd.dma_start(out=wzs[:, e, :], in_=wz[e, :, :])

            for j_rep in range(nch * repeat):
                j = j_rep % nch
                e, off, n = plan[j]
                xt = xpool.tile([P, KT, CHUNK], dt.bfloat16, tag="xt")
                if j_rep < 2:
                    # Fine-grained so the k=0 matmul starts ~5us earlier.
                    for k in range(KT):
                        nc.sync.dma_start(
                            out=xt[:, k, :n], in_=xc[j, :, k, :n]
                        )
                elif n == CHUNK:
                    nc.sync.dma_start(out=xt[:], in_=xc[j, :, :, :])
                else:
                    nc.sync.dma_start(out=xt[:, :, :n], in_=xc[j, :, :, :n])
                ht = hpool.tile([P, FT, CHUNK], dt.bfloat16, tag="ht")
                for f in range(FT):
                    ph = ps1.tile([P, n], dt.float32, tag="ph")
                    for k in range(KT):
                        nc.tensor.matmul(
                            ph[:],
                            lhsT=wzs[:, e, k * FS + f * P : k * FS + (f + 1) * P],
                            rhs=xt[:, k, :n],
                            start=(k == 0),
                            stop=(k == KT - 1),
                        )
                    # silu(x) = x * sigmoid(x); HW Silu LUT set is broken on
                    # this runtime, so compose it.
                    sg = spool.tile([P, CHUNK], dt.float32, tag="sg")
                    nc.scalar.activation(
                        sg[:, :n], ph[:], mybir.ActivationFunctionType.Sigmoid
                    )
                    nc.vector.tensor_mul(ht[:, f, :n], sg[:, :n], ph[:])
                ot = opool.tile([P, HT, CHUNK], dt.bfloat16, tag="ot")
                for h in range(HT):
                    py = ps2.tile([P, n], dt.float32, tag="py")
                    for f in range(FT):
                        a = W1SZ + f * H + h * P
                        nc.tensor.matmul(
                            py[:],
                            lhsT=wzs[:, e, a : a + P],
                            rhs=ht[:, f, :n],
                            start=(f == 0),
                            stop=(f == FT - 1),
                        )
                    nc.vector.tensor_copy(ot[:, h, :n], py[:])
                if n == CHUNK:
                    nc.sync.dma_start(out=yc[j, :, :, :], in_=ot[:])
                else:
                    nc.sync.dma_start(out=yc[j, :, :, :n], in_=ot[:, :, :n])

    nc.compile()
    return nc


def _get_module(plan, repeat: int = 1):
    key = (plan, repeat)
    if key not in _module_cache:
        _module_cache[key] = _build_module(plan, repeat)
    return _module_cache[key]


def _prepare(x, Wg, w1, w2):
    """Host dispatch: routing, per-expert gather, per-core input maps."""
    x = np.ascontiguousarray(np.asarray(x, np.float32))
    Wg = np.asarray(Wg, np.float32)
    w1 = np.ascontiguousarray(np.asarray(w1, np.float32))
    w2 = np.ascontiguousarray(np.asarray(w2, np.float32))
    nt = x.shape[0]

    ti, rw = _routing(x, Wg)

    idx_list, gate_list = [], []
    for e in range(E):
        hit = ti == e                                   # [nt, 2]
        rows = np.nonzero(hit.any(axis=1))[0]
        g = np.where(hit[rows, 0], rw[rows, 0], rw[rows, 1]).astype(np.float32)
        idx_list.append(rows)
        gate_list.append(g)

    cnts_padded = [-(-len(r) // PAD) * PAD for r in idx_list]
    offs = np.concatenate([[0], np.cumsum(cnts_padded)])
    ctot = int(offs[-1])
    plan = _chunk_plan(cnts_padded)
    nch = len(plan)

    # Gather all experts' tokens, then lay out chunk-major: xc[j] = [P, KT, n].
    xg = np.zeros((ctot, H), np.float32)
    for e in range(E):
        xg[offs[e] : offs[e] + len(idx_list[e])] = x[idx_list[e]]
    xgT = np.ascontiguousarray(xg.T).astype(BF16).reshape(KT, P, ctot)
    xch = np.zeros((nch, P, KT, CHUNK), BF16)
    for j, (e, off, n) in enumerate(plan):
        xch[j, :, :, :n] = xgT[:, :, off : off + n].transpose(1, 0, 2)

    w1b = w1.astype(BF16)
    w2b = w2.astype(BF16)
    in_maps = []
    for c in range(NCORE):
        w1c = (
            w1b[:, :, c * FS : (c + 1) * FS]
            .reshape(E, KT, P, FS)
            .transpose(0, 2, 1, 3)
            .reshape(E, P, KT * FS)
        )
        w2c = (
            w2b[:, c * FS : (c + 1) * FS, :]
            .reshape(E, FT, P, H)
            .transpose(0, 2, 1, 3)
            .reshape(E, P, FT * H)
        )
        wzc = np.concatenate([w1c, w2c], axis=2)
        in_maps.append({"xc": xch, "wz": np.ascontiguousarray(wzc)})

    meta = dict(nt=nt, idx_list=idx_list, gate_list=gate_list, offs=offs,
                ctot=ctot, plan=plan)
    return in_maps, meta


def _combine(results, meta):
    """Sum the 8 cores' F-slice partials, then gate-weighted scatter-add."""
    plan = meta["plan"]
    nch = len(plan)
    ysumc = np.zeros((nch, P, HT, CHUNK), np.float32)
    for r in results:
        ysumc += r["yc"]
    y = np.zeros((meta["nt"], H), np.float32)
    offs = meta["offs"]
    # Rebuild [H, Ctot] from chunk-major partial sums.
    ysum = np.empty((H, meta["ctot"]), np.float32)
    for j, (e, off, n) in enumerate(plan):
        ysum[:, off : off + n] = (
            ysumc[j, :, :, :n].transpose(1, 0, 2).reshape(H, n)
        )
    for e in range(E):
        rows = meta["idx_list"][e]
        cols = ysum[:, offs[e] : offs[e] + len(rows)]
        y[rows] += meta["gate_list"][e][:, None] * cols.T
    return y


def kernel(x: np.ndarray, Wg: np.ndarray, w1: np.ndarray, w2: np.ndarray,
           **_unused) -> np.ndarray:
    from concourse.bass_utils import run_bass_kernel_spmd

    in_maps, meta = _prepare(x, Wg, w1, w2)
    nc = _get_module(meta["plan"])
    res = run_bass_kernel_spmd(nc, in_maps, core_ids=list(range(NCORE)))
    return _combine(res.results, meta)


if __name__ == "__main__":
    rng = np.random.default_rng(0)
    xs = rng.standard_normal((T, H), dtype=np.float32)
    Wgs = rng.standard_normal((H, E), dtype=np.float32) / np.sqrt(H)
    w1s = rng.standard_normal((E, H, F), dtype=np.float32) / np.sqrt(H)
    w2s = rng.standard_normal((E, F, H), dtype=np.float32) / np.sqrt(F)
    out = kernel(x=xs, Wg=Wgs, w1=w1s, w2=w2s)
    print(out.shape, out.dtype)


# revision 10
# speedup vs baseline: 1.0418x; 1.0418x over previous
"""MoE layer (T=16384, H=1024, F=4096, E=8, top-2) on 8 Trainium2 cores.

Strategy (FFN-dim sharding — perfectly balanced):
  - Router (x @ Wg, softmax, top-2, renormalize) runs on host with jax-on-CPU
    so expert selection matches the reference bit-for-bit.
  - Tokens are gathered per expert into one concatenated sequence of
    column chunks (<=512 tokens, each chunk belonging to one expert, counts
    padded to a multiple of 4). The SAME token data ships to all 8 cores in
    a chunk-major layout xc[j] = [P, KT, n_j] so each chunk is ONE
    contiguous 1MB DMA (8KB per partition line).
  - Core c owns the F-slice [c*512, (c+1)*512) of ALL experts' w1/w2
    (16.8 MB bf16, SBUF-resident; preloaded on the scalar HWDGE ring so the
    token stream on the sync ring is not blocked behind it). It computes
        y_partial = silu(xe @ w1[e][:, sl]) @ w2[e][sl, :]
    for every chunk. Every core processes every token-expert pair ->
    identical work on all cores (zero load imbalance).
  - Host combine: y = sum over cores of partials (fp32), then scatter-add
    with the top-2 gate weights.
"""

import numpy as np
import ml_dtypes

T, H, F, E, TOPK = 16384, 1024, 4096, 8, 2
P = 128
NCORE = 8
FS = F // NCORE       # 512: per-core F-slice width
KT = H // P           # 8  k-tiles over H (GEMM1 contraction)
FT = FS // P          # 4  f-tiles per core slice
HT = H // P           # 8  output tiles over H
CHUNK = 512           # max matmul moving free dim (tokens per chunk)
PAD = 4               # per-expert token count padded to a multiple of this

BF16 = ml_dtypes.bfloat16

_module_cache: dict = {}


def _routing(x: np.ndarray, Wg: np.ndarray):
    """Top-2 expert ids and renormalized gates, matching the jax reference.

    The reference receives numpy arrays, so its `x @ Wg` runs through numpy
    BLAS — replicate that exactly (the expert ranking has 1-ulp knife-edge
    ties that flip between BLAS and XLA matmul). softmax/top_k then follow
    the reference's jax ops on CPU.
    """
    logits = x @ Wg  # numpy BLAS fp32, same as reference(**np_inputs)
    try:
        import jax
        import jax.numpy as jnp

        cpu = jax.devices("cpu")[0]
        with jax.default_device(cpu):
            lj = jax.device_put(jnp.asarray(logits), cpu)
            probs = jax.nn.softmax(lj, axis=-1)
            tv, ti = jax.lax.top_k(probs, TOPK)
            rw = tv / jnp.sum(tv, axis=-1, keepdims=True)
        return np.asarray(ti), np.asarray(rw, np.float32)
    except Exception:
        m = logits.max(axis=1, keepdims=True)
        p = np.exp(logits - m)
        p /= p.sum(axis=1, keepdims=True)
        order = np.argsort(-p, axis=1, kind="stable")
        ti = order[:, :TOPK]
        tv = np.take_along_axis(p, ti, axis=1)
        rw = (tv / tv.sum(axis=1, keepdims=True)).astype(np.float32)
        return ti, rw


def _chunk_plan(cnts_padded):
    """[(expert, col offset, n tokens)] covering each expert's padded range."""
    plan = []
    off = 0
    for e, c in enumerate(cnts_padded):
        rem = c
        while rem > 0:
            n = min(CHUNK, rem)
            plan.append((e, off, n))
            off += n
            rem -= n
    return tuple(plan)


def _build_module(plan, repeat: int = 1):
    """Bass/Tile module for one F-slice core.

    All weights (every expert's slice) are SBUF-resident; the token chunk
    loop runs GEMM1 (4 f-tiles x 8 k) -> silu -> GEMM2 (8 h-tiles x 4 f)
    with the chunk's expert baked in at compile time.
    """
    import concourse.bass as bass
    import concourse.mybir as mybir
    import concourse.tile as tile
    from concourse import bacc
    from concourse.bass import ts

    dt = mybir.dt
    nch = len(plan)

    nc = bacc.Bacc("TRN2", target_bir_lowering=False, debug=False)

    xc = nc.dram_tensor("xc", (nch, P, KT, CHUNK), dt.bfloat16,
                        kind="ExternalInput").ap()
    w1 = nc.dram_tensor("w1", (E, P, KT, FS), dt.bfloat16,
                        kind="ExternalInput").ap()
    w2 = nc.dram_tensor("w2", (E, P, FT, H), dt.bfloat16,
                        kind="ExternalInput").ap()
    yc = nc.dram_tensor("yc", (nch, P, HT, CHUNK), dt.bfloat16,
                        kind="ExternalOutput").ap()

    with tile.TileContext(nc) as tc:
        with (
            tc.tile_pool(name="wpool", bufs=1) as wpool,
            tc.tile_pool(name="xpool", bufs=3) as xpool,
            tc.tile_pool(name="hpool", bufs=2) as hpool,
            tc.tile_pool(name="opool", bufs=2) as opool,
            tc.tile_pool(name="spool", bufs=2) as spool,
            tc.tile_pool(name="ps1", bufs=4, space="PSUM") as ps1,
            tc.tile_pool(name="ps2", bufs=4, space="PSUM") as ps2,
        ):
            # Resident weights: 64KB + 64KB per partition. All weight DMAs
            # ride the sync ring, interleaved with the token chunks in
            # first-use order. Keeping them off the scalar engine matters:
            # DMA trigger instructions block that engine's queue on
            # semaphore-lane reuse, which would delay the first sigmoid
            # (and with it GEMM2) by ~25us. Expert 0 is split per k-tile
            # and alternated with chunk 0's x pieces so the first matmul
            # starts as soon as 128KB of weights + 128KB of x have landed;
            # experts 1..7 stream in half-an-expert per chunk over chunks
            # 1..14, well ahead of their first use (~8.5 chunks/expert).
            w1s = wpool.tile([P, E, KT, FS], dt.bfloat16)
            w2s = wpool.tile([P, E, FT, H], dt.bfloat16)
            wload: dict[int, list] = {}
            for e in range(1, E):
                wload.setdefault(min(2 * e - 1, nch - 1), []).append(("w1", e))
                wload.setdefault(min(2 * e, nch - 1), []).append(("w2", e))

            for j_rep in range(nch * repeat):
                j = j_rep % nch
                e, off, n = plan[j]
                xt = xpool.tile([P, KT, CHUNK], dt.bfloat16, tag="xt")
                if j_rep == 0:
                    # Interleave chunk-0 x pieces with expert-0 w1 k-tiles.
                    for k in range(KT):
                        nc.sync.dma_start(out=xt[:, k, :n], in_=xc[j, :, k, :n])
                        nc.sync.dma_start(
                            out=w1s[:, 0, k, :], in_=w1[0, :, k, :]
                        )
                    for f in range(FT):
                        nc.sync.dma_start(
                            out=w2s[:, 0, f, :], in_=w2[0, :, f, :]
                        )
                else:
                    if j_rep < nch:
                        for kind, we in wload.get(j, ()):
                            if kind == "w1":
                                nc.sync.dma_start(
                                    out=w1s[:, we, :, :], in_=w1[we, :, :, :]
                                )
                            else:
                                nc.sync.dma_start(
                                    out=w2s[:, we, :, :], in_=w2[we, :, :, :]
                                )
                    if n == CHUNK:
                        nc.sync.dma_start(out=xt[:], in_=xc[j, :, :, :])
                    else:
                        nc.sync.dma_start(out=xt[:, :, :n], in_=xc[j, :, :, :n])
                ht = hpool.tile([P, FT, CHUNK], dt.bfloat16, tag="ht")
                for f in range(FT):
                    ph = ps1.tile([P, n], dt.float32, tag="ph")
                    for k in range(KT):
                        nc.tensor.matmul(
                            ph[:],
                            lhsT=w1s[:, e, k, ts(f, P)],
                            rhs=xt[:, k, :n],
                            start=(k == 0),
                            stop=(k == KT - 1),
                        )
                    # silu(x) = x * sigmoid(x); HW Silu LUT set is broken on
                    # this runtime, so compose it.
                    sg = spool.tile([P, CHUNK], dt.float32, tag="sg")
                    nc.scalar.activation(
                        sg[:, :n], ph[:], mybir.ActivationFunctionType.Sigmoid
                    )
                    nc.vector.tensor_mul(ht[:, f, :n], sg[:, :n], ph[:])
                ot = opool.tile([P, HT, CHUNK], dt.bfloat16, tag="ot")
                for h in range(HT):
                    py = ps2.tile([P, n], dt.float32, tag="py")
                    for f in range(FT):
                        nc.tensor.matmul(
                            py[:],
                            lhsT=w2s[:, e, f, ts(h, P)],
                            rhs=ht[:, f, :n],
                            start=(f == 0),
                            stop=(f == FT - 1),
                        )
                    nc.vector.tensor_copy(ot[:, h, :n], py[:])
                if n == CHUNK:
                    nc.sync.dma_start(out=yc[j, :, :, :], in_=ot[:])
                else:
                    nc.sync.dma_start(out=yc[j, :, :, :n], in_=ot[:, :, :n])

    nc.compile()
    return nc


def _get_module(plan, repeat: int = 1):
    key = (plan, repeat)
    if key not in _module_cache:
        _module_cache[key] = _build_module(plan, repeat)
    return _module_cache[key]


def _prepare(x, Wg, w1, w2):
    """Host dispatch: routing, per-expert gather, per-core input maps."""
    x = np.ascontiguousarray(np.asarray(x, np.float32))
    Wg = np.asarray(Wg, np.float32)
    w1 = np.ascontiguousarray(np.asarray(w1, np.float32))
    w2 = np.ascontiguousarray(np.asarray(w2, np.float32))
    nt = x.shape[0]

    ti, rw = _routing(x, Wg)

    idx_list, gate_list = [], []
    for e in range(E):
        hit = ti == e                                   # [nt, 2]
        rows = np.nonzero(hit.any(axis=1))[0]
        g = np.where(hit[rows, 0], rw[rows, 0], rw[rows, 1]).astype(np.float32)
        idx_list.append(rows)
        gate_list.append(g)

    cnts_padded = [-(-len(r) // PAD) * PAD for r in idx_list]
    offs = np.concatenate([[0], np.cumsum(cnts_padded)])
    ctot = int(offs[-1])
    plan = _chunk_plan(cnts_padded)
    nch = len(plan)

    # Gather all experts' tokens, then lay out chunk-major: xc[j] = [P, KT, n].
    xg = np.zeros((ctot, H), np.float32)
    for e in range(E):
        xg[offs[e] : offs[e] + len(idx_list[e])] = x[idx_list[e]]
    xgT = np.ascontiguousarray(xg.T).astype(BF16).reshape(KT, P, ctot)
    xch = np.zeros((nch, P, KT, CHUNK), BF16)
    for j, (e, off, n) in enumerate(plan):
        xch[j, :, :, :n] = xgT[:, :, off : off + n].transpose(1, 0, 2)

    w1b = w1.astype(BF16)
    w2b = w2.astype(BF16)
    in_maps = []
    for c in range(NCORE):
        w1c = (
            w1b[:, :, c * FS : (c + 1) * FS]
            .reshape(E, KT, P, FS)
            .transpose(0, 2, 1, 3)
        )
        w2c = (
            w2b[:, c * FS : (c + 1) * FS, :]
            .reshape(E, FT, P, H)
            .transpose(0, 2, 1, 3)
        )
        in_maps.append(
            {
                "xc": xch,
                "w1": np.ascontiguousarray(w1c),
                "w2": np.ascontiguousarray(w2c),
            }
        )

    meta = dict(nt=nt, idx_list=idx_list, gate_list=gate_list, offs=offs,
                ctot=ctot, plan=plan)
    return in_maps, meta


def _combine(results, meta):
    """Sum the 8 cores' F-slice partials, then gate-weighted scatter-add."""
    plan = meta["plan"]
    nch = len(plan)
    ysumc = np.zeros((nch, P, HT, CHUNK), np.float32)
    for r in results:
        ysumc += r["yc"]
    y = np.zeros((meta["nt"], H), np.float32)
    offs = meta["offs"]
    # Rebuild [H, Ctot] from chunk-major partial sums.
    ysum = np.empty((H, meta["ctot"]), np.float32)
    for j, (e, off, n) in enumerate(plan):
        ysum[:, off : off + n] = (
            ysumc[j, :, :, :n].transpose(1, 0, 2).reshape(H, n)
        )
    for e in range(E):
        rows = meta["idx_list"][e]
        cols = ysum[:, offs[e] : offs[e] + len(rows)]
        y[rows] += meta["gate_list"][e][:, None] * cols.T
    return y


def kernel(x: np.ndarray, Wg: np.ndarray, w1: np.ndarray, w2: np.ndarray,
           **_unused) -> np.ndarray:
    from concourse.bass_utils import run_bass_kernel_spmd

    in_maps, meta = _prepare(x, Wg, w1, w2)
    nc = _get_module(meta["plan"])
    res = run_bass_kernel_spmd(nc, in_maps, core_ids=list(range(NCORE)))
    return _combine(res.results, meta)


if __name__ == "__main__":
    rng = np.random.default_rng(0)
    xs = rng.standard_normal((T, H), dtype=np.float32)
    Wgs = rng.standard_normal((H, E), dtype=np.float32) / np.sqrt(H)
    w1s = rng.standard_normal((E, H, F), dtype=np.float32) / np.sqrt(H)
    w2s = rng.standard_normal((E, F, H), dtype=np.float32) / np.sqrt(F)
    out = kernel(x=xs, Wg=Wgs, w1=w1s, w2=w2s)
    print(out.shape, out.dtype)


# revision 13
# speedup vs baseline: 1.0442x; 1.0023x over previous
"""MoE layer (T=16384, H=1024, F=4096, E=8, top-2) on 8 Trainium2 cores.

Strategy (FFN-dim sharding — perfectly balanced):
  - Router (x @ Wg, softmax, top-2, renormalize) runs on host with jax-on-CPU
    so expert selection matches the reference bit-for-bit.
  - Tokens are gathered per expert into one concatenated sequence of
    column chunks (<=512 tokens, each chunk belonging to one expert, counts
    padded to a multiple of 4). The SAME token data ships to all 8 cores in
    a chunk-major layout xc[j] = [P, KT, n_j] so each chunk is ONE
    contiguous 1MB DMA (8KB per partition line).
  - Core c owns the F-slice [c*512, (c+1)*512) of ALL experts' w1/w2
    (16.8 MB bf16, SBUF-resident; preloaded on the scalar HWDGE ring so the
    token stream on the sync ring is not blocked behind it). It computes
        y_partial = silu(xe @ w1[e][:, sl]) @ w2[e][sl, :]
    for every chunk. Every core processes every token-expert pair ->
    identical work on all cores (zero load imbalance).
  - Host combine: y = sum over cores of partials (fp32), then scatter-add
    with the top-2 gate weights.
"""

import numpy as np
import ml_dtypes

T, H, F, E, TOPK = 16384, 1024, 4096, 8, 2
P = 128
NCORE = 8
FS = F // NCORE       # 512: per-core F-slice width
KT = H // P           # 8  k-tiles over H (GEMM1 contraction)
FT = FS // P          # 4  f-tiles per core slice
HT = H // P           # 8  output tiles over H
CHUNK = 512           # max matmul moving free dim (tokens per chunk)
PAD = 4               # per-expert token count padded to a multiple of this

BF16 = ml_dtypes.bfloat16

_module_cache: dict = {}


def _routing(x: np.ndarray, Wg: np.ndarray):
    """Top-2 expert ids and renormalized gates, matching the jax reference.

    The reference receives numpy arrays, so its `x @ Wg` runs through numpy
    BLAS — replicate that exactly (the expert ranking has 1-ulp knife-edge
    ties that flip between BLAS and XLA matmul). softmax/top_k then follow
    the reference's jax ops on CPU.
    """
    logits = x @ Wg  # numpy BLAS fp32, same as reference(**np_inputs)
    try:
        import jax
        import jax.numpy as jnp

        cpu = jax.devices("cpu")[0]
        with jax.default_device(cpu):
            lj = jax.device_put(jnp.asarray(logits), cpu)
            probs = jax.nn.softmax(lj, axis=-1)
            tv, ti = jax.lax.top_k(probs, TOPK)
            rw = tv / jnp.sum(tv, axis=-1, keepdims=True)
        return np.asarray(ti), np.asarray(rw, np.float32)
    except Exception:
        m = logits.max(axis=1, keepdims=True)
        p = np.exp(logits - m)
        p /= p.sum(axis=1, keepdims=True)
        order = np.argsort(-p, axis=1, kind="stable")
        ti = order[:, :TOPK]
        tv = np.take_along_axis(p, ti, axis=1)
        rw = (tv / tv.sum(axis=1, keepdims=True)).astype(np.float32)
        return ti, rw


def _chunk_plan(cnts_padded):
    """[(expert, col offset, n tokens)] covering each expert's padded range."""
    plan = []
    off = 0
    for e, c in enumerate(cnts_padded):
        rem = c
        while rem > 0:
            n = min(CHUNK, rem)
            plan.append((e, off, n))
            off += n
            rem -= n
    return tuple(plan)


def _build_module(plan, repeat: int = 1):
    """Bass/Tile module for one F-slice core.

    All weights (every expert's slice) are SBUF-resident; the token chunk
    loop runs GEMM1 (4 f-tiles x 8 k) -> silu -> GEMM2 (8 h-tiles x 4 f)
    with the chunk's expert baked in at compile time.
    """
    import concourse.bass as bass
    import concourse.mybir as mybir
    import concourse.tile as tile
    from concourse import bacc
    from concourse.bass import ts

    dt = mybir.dt
    nch = len(plan)

    nc = bacc.Bacc("TRN2", target_bir_lowering=False, debug=False)

    xc = nc.dram_tensor("xc", (nch, P, KT, CHUNK), dt.bfloat16,
                        kind="ExternalInput").ap()
    w1 = nc.dram_tensor("w1", (E, P, KT, FS), dt.bfloat16,
                        kind="ExternalInput").ap()
    w2 = nc.dram_tensor("w2", (E, P, FT, H), dt.bfloat16,
                        kind="ExternalInput").ap()
    yc = nc.dram_tensor("yc", (nch, P, HT, CHUNK), dt.bfloat16,
                        kind="ExternalOutput").ap()

    with tile.TileContext(nc) as tc:
        with (
            tc.tile_pool(name="wpool", bufs=1) as wpool,
            tc.tile_pool(name="xpool", bufs=3) as xpool,
            tc.tile_pool(name="hpool", bufs=2) as hpool,
            tc.tile_pool(name="opool", bufs=2) as opool,
            tc.tile_pool(name="spool", bufs=2) as spool,
            tc.tile_pool(name="ps1", bufs=4, space="PSUM") as ps1,
            tc.tile_pool(name="ps2", bufs=4, space="PSUM") as ps2,
        ):
            # Resident weights: 64KB + 64KB per partition. All weight DMAs
            # ride the sync ring, interleaved with the token chunks in
            # first-use order. Keeping them off the scalar engine matters:
            # DMA trigger instructions block that engine's queue on
            # semaphore-lane reuse, which would delay the first sigmoid
            # (and with it GEMM2) by ~25us. Expert 0 is split per k-tile
            # and alternated with chunk 0's x pieces so the first matmul
            # starts as soon as 128KB of weights + 128KB of x have landed;
            # experts 1..7 stream in half-an-expert per chunk over chunks
            # 1..14, well ahead of their first use (~8.5 chunks/expert).
            w1s = wpool.tile([P, E, KT, FS], dt.bfloat16)
            w2s = wpool.tile([P, E, FT, H], dt.bfloat16)
            # Quarter-expert pieces (~1MB), one per chunk from chunk 1 on:
            # uniform ring load (x 1MB + y 0.5MB + w 1MB < chunk compute
            # time), each expert complete by chunk 4e << first use ~8.5e.
            wload: dict[int, list] = {}
            pieces = []
            for e in range(1, E):
                pieces += [("w1a", e), ("w1b", e), ("w2a", e), ("w2b", e)]
            for i, pc in enumerate(pieces):
                wload.setdefault(min(i + 1, nch - 1), []).append(pc)

            for j_rep in range(nch * repeat):
                j = j_rep % nch
                e, off, n = plan[j]
                xt = xpool.tile([P, KT, CHUNK], dt.bfloat16, tag="xt")
                if j_rep == 0:
                    # Interleave chunk-0 x pieces with expert-0 w1 k-tiles.
                    for k in range(KT):
                        nc.sync.dma_start(out=xt[:, k, :n], in_=xc[j, :, k, :n])
                        nc.sync.dma_start(
                            out=w1s[:, 0, k, :], in_=w1[0, :, k, :]
                        )
                    for f in range(FT):
                        nc.sync.dma_start(
                            out=w2s[:, 0, f, :], in_=w2[0, :, f, :]
                        )
                else:
                    if j_rep < nch:
                        for kind, we in wload.get(j, ()):
                            if kind == "w1a":
                                nc.sync.dma_start(
                                    out=w1s[:, we, :4, :], in_=w1[we, :, :4, :]
                                )
                            elif kind == "w1b":
                                nc.sync.dma_start(
                                    out=w1s[:, we, 4:, :], in_=w1[we, :, 4:, :]
                                )
                            elif kind == "w2a":
                                nc.sync.dma_start(
                                    out=w2s[:, we, :2, :], in_=w2[we, :, :2, :]
                                )
                            else:
                                nc.sync.dma_start(
                                    out=w2s[:, we, 2:, :], in_=w2[we, :, 2:, :]
                                )
                    if n == CHUNK:
                        nc.sync.dma_start(out=xt[:], in_=xc[j, :, :, :])
                    else:
                        nc.sync.dma_start(out=xt[:, :, :n], in_=xc[j, :, :, :n])
                ht = hpool.tile([P, FT, CHUNK], dt.bfloat16, tag="ht")
                for f in range(FT):
                    ph = ps1.tile([P, n], dt.float32, tag="ph")
                    for k in range(KT):
                        nc.tensor.matmul(
                            ph[:],
                            lhsT=w1s[:, e, k, ts(f, P)],
                            rhs=xt[:, k, :n],
                            start=(k == 0),
                            stop=(k == KT - 1),
                        )
                    # silu(x) = x * sigmoid(x); HW Silu LUT set is broken on
                    # this runtime, so compose it.
                    sg = spool.tile([P, CHUNK], dt.float32, tag="sg")
                    nc.scalar.activation(
                        sg[:, :n], ph[:], mybir.ActivationFunctionType.Sigmoid
                    )
                    nc.vector.tensor_mul(ht[:, f, :n], sg[:, :n], ph[:])
                last = j_rep == nch * repeat - 1
                ot = opool.tile([P, HT, CHUNK], dt.bfloat16, tag="ot")
                for h in range(HT):
                    py = ps2.tile([P, n], dt.float32, tag="py")
                    for f in range(FT):
                        nc.tensor.matmul(
                            py[:],
                            lhsT=w2s[:, e, f, ts(h, P)],
                            rhs=ht[:, f, :n],
                            start=(f == 0),
                            stop=(f == FT - 1),
                        )
                    nc.vector.tensor_copy(ot[:, h, :n], py[:])
                    if last:
                        # Per-h stores so the final DMA chain starts as
                        # soon as each copy lands (shorter kernel drain).
                        nc.sync.dma_start(
                            out=yc[j, :, h, :n], in_=ot[:, h, :n]
                        )
                if not last:
                    if n == CHUNK:
                        nc.sync.dma_start(out=yc[j, :, :, :], in_=ot[:])
                    else:
                        nc.sync.dma_start(
                            out=yc[j, :, :, :n], in_=ot[:, :, :n]
                        )

    nc.compile()
    return nc


def _get_module(plan, repeat: int = 1):
    key = (plan, repeat)
    if key not in _module_cache:
        _module_cache[key] = _build_module(plan, repeat)
    return _module_cache[key]


def _prepare(x, Wg, w1, w2):
    """Host dispatch: routing, per-expert gather, per-core input maps."""
    x = np.ascontiguousarray(np.asarray(x, np.float32))
    Wg = np.asarray(Wg, np.float32)
    w1 = np.ascontiguousarray(np.asarray(w1, np.float32))
    w2 = np.ascontiguousarray(np.asarray(w2, np.float32))
    nt = x.shape[0]

    ti, rw = _routing(x, Wg)

    idx_list, gate_list = [], []
    for e in range(E):
        hit = ti == e                                   # [nt, 2]
        rows = np.nonzero(hit.any(axis=1))[0]
        g = np.where(hit[rows, 0], rw[rows, 0], rw[rows, 1]).astype(np.float32)
        idx_list.append(rows)
        gate_list.append(g)

    cnts_padded = [-(-len(r) // PAD) * PAD for r in idx_list]
    offs = np.concatenate([[0], np.cumsum(cnts_padded)])
    ctot = int(offs[-1])
    plan = _chunk_plan(cnts_padded)
    nch = len(plan)

    # Gather all experts' tokens, then lay out chunk-major: xc[j] = [P, KT, n].
    xg = np.zeros((ctot, H), np.float32)
    for e in range(E):
        xg[offs[e] : offs[e] + len(idx_list[e])] = x[idx_list[e]]
    xgT = np.ascontiguousarray(xg.T).astype(BF16).reshape(KT, P, ctot)
    xch = np.zeros((nch, P, KT, CHUNK), BF16)
    for j, (e, off, n) in enumerate(plan):
        xch[j, :, :, :n] = xgT[:, :, off : off + n].transpose(1, 0, 2)

    w1b = w1.astype(BF16)
    w2b = w2.astype(BF16)
    in_maps = []
    for c in range(NCORE):
        w1c = (
            w1b[:, :, c * FS : (c + 1) * FS]
            .reshape(E, KT, P, FS)
            .transpose(0, 2, 1, 3)
        )
        w2c = (
            w2b[:, c * FS : (c + 1) * FS, :]
            .reshape(E, FT, P, H)
            .transpose(0, 2, 1, 3)
        )
        in_maps.append(
            {
                "xc": xch,
                "w1": np.ascontiguousarray(w1c),
                "w2": np.ascontiguousarray(w2c),
            }
        )

    meta = dict(nt=nt, idx_list=idx_list, gate_list=gate_list, offs=offs,
                ctot=ctot, plan=plan)
    return in_maps, meta


def _combine(results, meta):
    """Sum the 8 cores' F-slice partials, then gate-weighted scatter-add."""
    plan = meta["plan"]
    nch = len(plan)
    ysumc = np.zeros((nch, P, HT, CHUNK), np.float32)
    for r in results:
        ysumc += r["yc"]
    y = np.zeros((meta["nt"], H), np.float32)
    offs = meta["offs"]
    # Rebuild [H, Ctot] from chunk-major partial sums.
    ysum = np.empty((H, meta["ctot"]), np.float32)
    for j, (e, off, n) in enumerate(plan):
        ysum[:, off : off + n] = (
            ysumc[j, :, :, :n].transpose(1, 0, 2).reshape(H, n)
        )
    for e in range(E):
        rows = meta["idx_list"][e]
        cols = ysum[:, offs[e] : offs[e] + len(rows)]
        y[rows] += meta["gate_list"][e][:, None] * cols.T
    return y


def kernel(x: np.ndarray, Wg: np.ndarray, w1: np.ndarray, w2: np.ndarray,
           **_unused) -> np.ndarray:
    from concourse.bass_utils import run_bass_kernel_spmd

    in_maps, meta = _prepare(x, Wg, w1, w2)
    nc = _get_module(meta["plan"])
    res = run_bass_kernel_spmd(nc, in_maps, core_ids=list(range(NCORE)))
    return _combine(res.results, meta)


if __name__ == "__main__":
    rng = np.random.default_rng(0)
    xs = rng.standard_normal((T, H), dtype=np.float32)
    Wgs = rng.standard_normal((H, E), dtype=np.float32) / np.sqrt(H)
    w1s = rng.standard_normal((E, H, F), dtype=np.float32) / np.sqrt(H)
    w2s = rng.standard_normal((E, F, H), dtype=np.float32) / np.sqrt(F)
    out = kernel(x=xs, Wg=Wgs, w1=w1s, w2=w2s)
    print(out.shape, out.dtype)
